# revision 6
# baseline (speedup 1.0000x reference)
"""Trainium2 Bass kernel for nn_Erode (5x5 all-ones SE, zero padding).

For an all-ones 5x5 structuring element, kornia-style Erode reduces to a
5x5 sliding-window MIN over the zero-padded image.  The min is separable
(vertical 5-tap then horizontal 5-tap), each direction done with 3
tensor_tensor(min) ops on the Vector engine.

Precision: the harness tolerance is rel_err < 2e-2; fp16 quantization of
the inputs costs ~5e-4, so the whole kernel runs in fp16.  That halves
DMA bytes AND doubles DVE throughput: fp16 tensor_tensor runs in the
2x_1p perf mode (2 elem/cycle/lane) provided every operand is 4-byte
aligned with unit stride.  All vertical shifts are whole row-slots (even
element counts -> aligned).  The horizontal 5-tap is decomposed into
even shifts only:  A = min(V, V<<2), B = min(A, A<<2) (covers shifts
{0,2,4}), and the odd-parity part A<<1 (covers {1,3}) is materialized by
a small SBUF->SBUF DMA copy (byte-addressed, no alignment limits), so
the final op  out = min(B, copy(A<<1))  also runs at 2x.  For the last
(small) chunk the copy latency can't hide, so it uses the direct
unaligned 1x op instead.

Distribution: pure data parallel.  B*C = 24 images of 512x512 are split
3-per-core across 8 NeuronCores.  Inside a core, the 3 images' rows are
striped over SBUF partitions: partition p = 40*i + j owns K=13 output
rows of image i (engine ops cannot read partition-shifted operands, so
each partition receives its rows plus a 2-row halo as 17 free-dim
row-slots, making both min passes pure free-dim sliding ops).  8 junk
stripes pad the partition count to 128 (DVE time depends only on the
free-dim size, and full-width DMAs are faster).

The HOST converts to fp16, pre-gathers the stripes (zero-padded, halos
duplicated, column-chunked) so every device DMA is a large contiguous-
per-partition transfer, and un-stripes/up-converts the output.  Columns
are processed in 6 chunks with ramped widths: small leading chunks whose
input rides the low-latency HWDGE queues (sync/scalar) get the Vector
engine computing ~3.5us into the kernel; mid-size trailing chunks keep
the store tail short.  Later chunk loads ride SWDGE (gpsimd) to keep the
HWDGE rings free for stores and the A<<1 copies (HWDGE is FIFO per
queue, so a store waiting on compute must not sit ahead of a load).
final(ch-1) is emitted after B(ch) so each A<<1 copy has a full chunk of
DVE work to hide behind.
"""

import numpy as np

# ---- fixed problem geometry (hardcoded per harness contract) ----
B, C, H, W = 8, 3, 512, 512
N_CORES = 8
IMGS = (B * C) // N_CORES  # 3 images per core
K = 13                   # output rows per partition
SLOTS = K + 4            # row-slots incl. 2+2 halo
PPI = 40                 # partitions per image = ceil(512/13)
NP = 128                 # DMA/compute partition width (8 junk stripes padded)
NP_DATA = IMGS * PPI     # 120 partitions carry real data
PAD_H = 2 + H + 10       # 524: top pad + data + tail pad (covers slot overrun)
PAD_W = 2 + W + 2        # 516
# ramped column chunks: small first chunks = short DMA ramp before the
# first vector op; smaller last chunk = short store drain after the last.
CWS = [16, 48, 96, 160, 128, 64]
NCH = len(CWS)
LWS = [cw + 4 for cw in CWS]
CHUNK_C0 = [sum(CWS[:i]) for i in range(NCH)]
H_SPLITS = [1, 1, 1, 1, 1, 2]          # final-op col pieces per chunk
SSPLIT = 9                             # slot split for 2-ring loads

IN_ELEMS = NP * SLOTS * sum(LWS)
OUT_ELEMS = NP * K * W

_cached = {}


def _build_program():
    import concourse.mybir as mybir
    from concourse import bass, bacc
    from concourse.tile import TileContext

    f16 = mybir.dt.float16
    MIN = mybir.AluOpType.min

    nc = bacc.Bacc("TRN2", target_bir_lowering=False, debug=False,
                   num_devices=N_CORES)
    xs = nc.dram_tensor("xs", [IN_ELEMS], f16, kind="ExternalInput")
    ys = nc.dram_tensor("ys", [OUT_ELEMS], f16, kind="ExternalOutput")

    with TileContext(nc) as tc:
        with tc.tile_pool(name="work", bufs=1) as pool:
            # (s0, s1, engine) load pieces per chunk: early chunks split
            # across the two HWDGE rings (in-ring FIFO order = chunk
            # order, so early chunks finish first), late chunks on SWDGE.
            two_ring = [(0, SSPLIT, nc.sync), (SSPLIT, SLOTS, nc.scalar)]
            load_plan = [
                two_ring, two_ring, two_ring, two_ring,
                [(0, SLOTS, nc.gpsimd)],
                [(0, SLOTS, nc.gpsimd)],
            ]
            # A<<1 copy engine per chunk.  Only the big middle chunks are
            # worth a copy (DVE saving 6.5*cw cycles); small chunks use
            # the direct unaligned 1x final instead.
            a1_eng = [None, None, nc.sync, nc.scalar, nc.sync, None]

            # ---- phase 1: all input loads (no deps; keep rings clear) ----
            X = []
            in_off = 0
            for ch in range(NCH):
                lw = LWS[ch]
                Xt = pool.tile([NP, SLOTS, lw], f16, tag=f"X{ch}")
                X.append(Xt)
                for s0, s1, eng in load_plan[ch]:
                    src = bass.AP(
                        tensor=xs,
                        offset=in_off + s0 * lw,
                        ap=[[SLOTS * lw, NP], [lw, s1 - s0], [1, lw]],
                    )
                    eng.dma_start(out=Xt[:, s0:s1], in_=src)
                in_off += NP * SLOTS * lw

            # ---- phase 2: software-pipelined compute ----
            out_off = 0
            pend = None  # (ch, A, B, A1) awaiting final+store

            def emit_final(p):
                nonlocal out_off
                ch, A, Bt, A1r = p
                cw = CWS[ch]
                nsp = H_SPLITS[ch]
                bounds = [cw * t // nsp for t in range(nsp + 1)]
                for t in range(nsp):
                    b0, b1 = bounds[t], bounds[t + 1]
                    pw = b1 - b0
                    Hm = pool.tile([NP, K, pw], f16, tag=f"H{ch}_{t}")
                    in1 = A1r[:, :, b0:b1] if A1r is not None \
                        else A[:, :, 1 + b0:1 + b1]
                    nc.vector.tensor_tensor(out=Hm, in0=Bt[:, :, b0:b1],
                                            in1=in1, op=MIN)
                    kh = K // 2
                    for (v0, v1), eng in (((0, kh), nc.sync),
                                          ((kh, K), nc.scalar)):
                        dst = bass.AP(
                            tensor=ys,
                            offset=out_off + v0 * pw,
                            ap=[[K * pw, NP], [pw, v1 - v0], [1, pw]],
                        )
                        eng.dma_start(out=dst, in_=Hm[:, v0:v1])
                    out_off += NP * K * pw

            for ch in range(NCH):
                lw = LWS[ch]
                cw = CWS[ch]
                Xt = X[ch]
                # vertical 5-tap min along row-slots:
                # P[s] = min(X[s], X[s+1]); Q = min(P[0:K], X[4:]) and
                # V = min(Q, P[2:K+2]) give V[j] = min(X[j..j+4]).
                NSL = SLOTS - 2  # 15
                P = pool.tile([NP, NSL, lw], f16, tag=f"P{ch}")
                # split P at the load's slot boundary so the first piece
                # only waits on the first ring's half of the load
                psb = [0, SSPLIT - 1, NSL] if len(load_plan[ch]) > 1 \
                    else [0, NSL]
                for k in range(len(psb) - 1):
                    s0, s1 = psb[k], psb[k + 1]
                    nc.vector.tensor_tensor(
                        out=P[:, s0:s1], in0=Xt[:, s0:s1],
                        in1=Xt[:, s0 + 1:s1 + 1], op=MIN)
                Q = pool.tile([NP, K, lw], f16, tag=f"Q{ch}")
                nc.vector.tensor_tensor(out=Q, in0=P[:, 0:K],
                                        in1=Xt[:, 4:SLOTS], op=MIN)
                V = pool.tile([NP, K, lw], f16, tag=f"V{ch}")
                nc.vector.tensor_tensor(out=V, in0=Q,
                                        in1=P[:, 2:K + 2], op=MIN)

                # horizontal 5-tap min, even shifts only (keeps 2x mode):
                # A = min(V, V<<2); B = min(A, A<<2) covers {0,2,4}.
                A = pool.tile([NP, K, lw - 2], f16, tag=f"A{ch}")
                nc.vector.tensor_tensor(out=A, in0=V[:, :, 0:lw - 2],
                                        in1=V[:, :, 2:lw], op=MIN)
                Bt = pool.tile([NP, K, lw - 4], f16, tag=f"B{ch}")
                nc.vector.tensor_tensor(out=Bt, in0=A[:, :, 0:lw - 4],
                                        in1=A[:, :, 2:lw - 2], op=MIN)

                # odd-parity part {1,3} = A<<1, materialized 4B-aligned by
                # a byte-addressed DMA copy so the final op runs at 2x.
                # The copy is done flat (one contiguous run per partition,
                # junk in the last column of each row, which is never read).
                A1r = None
                if a1_eng[ch] is not None:
                    aw = lw - 2
                    A1 = pool.tile([NP, K * aw], f16, tag=f"A1{ch}")
                    L = K * aw - 1
                    Af = A.rearrange("p k w -> p (k w)")
                    a1_eng[ch].dma_start(out=A1[:, 0:L],
                                         in_=Af[:, 1:1 + L])
                    A1r = A1.rearrange("p (k w) -> p k w", k=K, w=aw)

                if pend is not None:
                    emit_final(pend)
                pend = (ch, A, Bt, A1r)
            emit_final(pend)
    nc.compile()
    return nc


def _get_program():
    if "nc" not in _cached:
        _cached["nc"] = _build_program()
    return _cached["nc"]


# stripe gather index: [PPI, SLOTS] padded-row index per (j, s)
_ROW_IDX = (K * np.arange(PPI)[:, None] + np.arange(SLOTS)[None, :])


def _stripe_core_input(x3: np.ndarray) -> np.ndarray:
    """[3,512,512] f16 -> host-striped flat input [IN_ELEMS] f16."""
    xp = np.zeros((IMGS, PAD_H, PAD_W), np.float16)
    xp[:, 2:2 + H, 2:2 + W] = x3
    stripes = np.zeros((NP, SLOTS, PAD_W), np.float16)
    stripes[:NP_DATA] = xp[:, _ROW_IDX, :].reshape(NP_DATA, SLOTS, PAD_W)
    parts = [
        stripes[:, :, c0:c0 + lw].reshape(-1)
        for c0, lw in zip(CHUNK_C0, LWS)
    ]
    return np.concatenate(parts)


def _out_pieces():
    pieces = []
    for ch in range(NCH):
        cw = CWS[ch]
        nsp = H_SPLITS[ch]
        bounds = [cw * t // nsp for t in range(nsp + 1)]
        for t in range(nsp):
            pieces.append((CHUNK_C0[ch] + bounds[t], bounds[t + 1] - bounds[t]))
    return pieces


_PIECES = None


def _unstripe_core_output(flat: np.ndarray) -> np.ndarray:
    """piece-blocked f16 output -> [3,512,512] f16."""
    global _PIECES
    if _PIECES is None:
        _PIECES = _out_pieces()
    stripes = np.empty((NP_DATA, K, W), np.float16)
    off = 0
    for col0, pw in _PIECES:
        blk = flat[off:off + NP * K * pw].reshape(NP, K, pw)
        stripes[:, :, col0:col0 + pw] = blk[:NP_DATA]
        off += NP * K * pw
    ys = stripes.reshape(IMGS, PPI, K, W)
    out = np.empty((IMGS, H, W), np.float16)
    full = (PPI - 1) * K  # 507 rows from full partitions
    out[:, :full] = ys[:, :PPI - 1].reshape(IMGS, full, W)
    out[:, full:] = ys[:, PPI - 1, :H - full]
    return out


def _run_on_hw(x24: np.ndarray, trace: bool = False):
    from concourse.bass_utils import run_bass_kernel_spmd
    nc = _get_program()
    x16 = x24.astype(np.float16)
    in_maps = [
        {"xs": _stripe_core_input(x16[IMGS * k:IMGS * (k + 1)])}
        for k in range(N_CORES)
    ]
    try:
        res = run_bass_kernel_spmd(nc, in_maps, list(range(N_CORES)),
                                   trace=trace)
    except Exception:
        import time
        time.sleep(5)
        res = run_bass_kernel_spmd(nc, in_maps, list(range(N_CORES)),
                                   trace=trace)
    out = np.stack([
        _unstripe_core_output(res.results[k]["ys"]) for k in range(N_CORES)
    ])
    return out.reshape(B, C, H, W).astype(np.float32), res


def _erode_reference_np(x: np.ndarray, se: np.ndarray) -> np.ndarray:
    """Generic fallback faithful to the kornia-style formula (numpy)."""
    kh, kw = se.shape
    ph, pw = kh // 2, kw // 2
    xpad = np.pad(x, ((0, 0), (0, 0), (ph, ph), (pw, pw)))
    out = None
    for r in range(kh):
        for c in range(kw):
            shifted = xpad[:, :, r:r + x.shape[2], c:c + x.shape[3]]
            bias = se[r, c] - 1.0
            val = shifted - bias if bias >= 0.0 else np.full_like(shifted, -bias)
            out = val if out is None else np.minimum(out, val)
    return out.astype(x.dtype)


def kernel(x, se):
    x = np.asarray(x, dtype=np.float32)
    se = np.asarray(se, dtype=np.float32)
    if se.shape != (5, 5) or not np.all(se == 1.0) or x.shape != (B, C, H, W):
        return _erode_reference_np(x, se)
    x24 = np.ascontiguousarray(x.reshape(B * C, H, W))
    out, _ = _run_on_hw(x24, trace=False)
    return out


# revision 15
# speedup vs baseline: 1.0945x; 1.0945x over previous
"""Trainium2 Bass kernel for nn_Erode (5x5 all-ones SE, zero padding).

For an all-ones 5x5 structuring element, kornia-style Erode reduces to a
5x5 sliding-window MIN over the zero-padded image.  The min is separable
(vertical 5-tap then horizontal 5-tap), each direction done with 3
tensor_tensor(min) ops on the Vector engine.

Precision: the harness tolerance is rel_err < 2e-2; fp16 quantization of
the inputs costs ~5e-4, so the whole kernel runs in fp16.  That halves
DMA bytes AND doubles DVE throughput: fp16 tensor_tensor runs in the
2x_1p perf mode (2 elem/cycle/lane) provided every operand is 4-byte
aligned with unit stride.  All vertical shifts are whole row-slots (even
element counts -> aligned).  The horizontal 5-tap is decomposed into
even shifts only:  A = min(V, V<<2), B = min(A, A<<2) (covers shifts
{0,2,4}), and the odd-parity part A<<1 (covers {1,3}) is materialized by
a small SBUF->SBUF DMA copy (byte-addressed, no alignment limits), so
the final op  out = min(B, copy(A<<1))  also runs at 2x.  For the last
(small) chunk the copy latency can't hide, so it uses the direct
unaligned 1x op instead.

Distribution: pure data parallel.  B*C = 24 images of 512x512 are split
3-per-core across 8 NeuronCores.  Inside a core, the 3 images' rows are
striped over SBUF partitions: partition p = 40*i + j owns K=13 output
rows of image i (engine ops cannot read partition-shifted operands, so
each partition receives its rows plus a 2-row halo as 17 free-dim
row-slots, making both min passes pure free-dim sliding ops).  8 junk
stripes pad the partition count to 128 (DVE time depends only on the
free-dim size, and full-width DMAs are faster).

The HOST converts to fp16, pre-gathers the stripes (zero-padded, halos
duplicated, column-chunked) so every device DMA is a large contiguous-
per-partition transfer, and un-stripes/up-converts the output.  Columns
are processed in 6 chunks with ramped widths: small leading chunks whose
input rides the low-latency HWDGE queues (sync/scalar) get the Vector
engine computing ~3.5us into the kernel; mid-size trailing chunks keep
the store tail short.  Later chunk loads ride SWDGE (gpsimd) to keep the
HWDGE rings free for stores and the A<<1 copies (HWDGE is FIFO per
queue, so a store waiting on compute must not sit ahead of a load).
final(ch-1) is emitted after B(ch) so each A<<1 copy has a full chunk of
DVE work to hide behind.
"""

import numpy as np

# ---- fixed problem geometry (hardcoded per harness contract) ----
B, C, H, W = 8, 3, 512, 512
N_CORES = 8
IMGS = (B * C) // N_CORES  # 3 images per core
K = 13                   # output rows per partition
SLOTS = K + 4            # row-slots incl. 2+2 halo
PPI = 40                 # partitions per image = ceil(512/13)
NP = 128                 # DMA/compute partition width (8 junk stripes padded)
NP_DATA = IMGS * PPI     # 120 partitions carry real data
PAD_H = 2 + H + 10       # 524: top pad + data + tail pad (covers slot overrun)
PAD_W = 2 + W + 2        # 516
# ramped column chunks: small first chunks = short DMA ramp before the
# first vector op; smaller last chunk = short store drain after the last.
CWS = [16, 32, 80, 176, 144, 64]
NCH = len(CWS)
LWS = [cw + 4 for cw in CWS]
CHUNK_C0 = [sum(CWS[:i]) for i in range(NCH)]
H_SPLITS = [1, 1, 1, 1, 1, 2]          # final-op col pieces per chunk
SSPLIT = 9                             # slot split for 2-ring loads

IN_ELEMS = NP * SLOTS * sum(LWS)
OUT_ELEMS = NP * K * W

_cached = {}


def _build_program():
    import concourse.mybir as mybir
    from concourse import bass, bacc
    from concourse.tile import TileContext

    f16 = mybir.dt.float16
    MIN = mybir.AluOpType.min

    nc = bacc.Bacc("TRN2", target_bir_lowering=False, debug=False,
                   num_devices=N_CORES)
    xs = nc.dram_tensor("xs", [IN_ELEMS], f16, kind="ExternalInput")
    ys = nc.dram_tensor("ys", [OUT_ELEMS], f16, kind="ExternalOutput")

    with TileContext(nc) as tc:
        with tc.tile_pool(name="work", bufs=1) as pool:
            # (s0, s1, engine) load pieces per chunk.  A HWDGE ring
            # serializes its dma_starts with a ~1.4us gap each, so only
            # chunk0 rides the two rings (lowest first-byte latency);
            # chunks 1-5 go on the SWDGE queue, whose descriptor
            # generation pipelines across dma_starts (~230 GB/s
            # sustained, back to back).
            two_ring = [(0, SSPLIT, nc.sync), (SSPLIT, SLOTS, nc.scalar)]
            load_plan = [
                two_ring,
                [(0, SLOTS, nc.gpsimd)],
                [(0, SLOTS, nc.gpsimd)],
                [(0, SLOTS, nc.gpsimd)],
                [(0, SLOTS, nc.gpsimd)],
                [(0, SLOTS, nc.gpsimd)],
            ]
            # A<<1 copy per chunk.  Only the big middle chunks are worth
            # a copy (DVE saving 6.5*cw cycles); small chunks use the
            # direct unaligned 1x final instead.  Copies ride the SWDGE
            # queue too: descgen pipelines (no ring serialization), all
            # loads are ready at t=0 so they dispatch ahead of the
            # copies' compute waits, and nothing else queues after.
            a1_q1 = [False, False, True, True, True, False]

            # ---- phase 1: all input loads (no deps; keep rings clear) ----
            X = []
            in_off = 0
            for ch in range(NCH):
                lw = LWS[ch]
                Xt = pool.tile([NP, SLOTS, lw], f16, tag=f"X{ch}")
                X.append(Xt)
                for s0, s1, eng in load_plan[ch]:
                    src = bass.AP(
                        tensor=xs,
                        offset=in_off + s0 * lw,
                        ap=[[SLOTS * lw, NP], [lw, s1 - s0], [1, lw]],
                    )
                    eng.dma_start(out=Xt[:, s0:s1], in_=src)
                in_off += NP * SLOTS * lw

            # ---- phase 2: software-pipelined compute ----
            out_off = 0
            pend = None  # (ch, A, B, A1) awaiting final+store

            def emit_final(p):
                nonlocal out_off
                ch, A, Bt, A1r = p
                cw = CWS[ch]
                nsp = H_SPLITS[ch]
                bounds = [cw * t // nsp for t in range(nsp + 1)]
                for t in range(nsp):
                    b0, b1 = bounds[t], bounds[t + 1]
                    pw = b1 - b0
                    Hm = pool.tile([NP, K, pw], f16, tag=f"H{ch}_{t}")
                    in1 = A1r[:, :, b0:b1] if A1r is not None \
                        else A[:, :, 1 + b0:1 + b1]
                    nc.vector.tensor_tensor(out=Hm, in0=Bt[:, :, b0:b1],
                                            in1=in1, op=MIN)
                    kh = K // 2
                    for (v0, v1), eng in (((0, kh), nc.sync),
                                          ((kh, K), nc.scalar)):
                        dst = bass.AP(
                            tensor=ys,
                            offset=out_off + v0 * pw,
                            ap=[[K * pw, NP], [pw, v1 - v0], [1, pw]],
                        )
                        eng.dma_start(out=dst, in_=Hm[:, v0:v1])
                    out_off += NP * K * pw

            for ch in range(NCH):
                lw = LWS[ch]
                cw = CWS[ch]
                Xt = X[ch]
                # vertical 5-tap min along row-slots:
                # P[s] = min(X[s], X[s+1]); Q = min(P[0:K], X[4:]) and
                # V = min(Q, P[2:K+2]) give V[j] = min(X[j..j+4]).
                NSL = SLOTS - 2  # 15
                P = pool.tile([NP, NSL, lw], f16, tag=f"P{ch}")
                # split P at the load's slot boundary so the first piece
                # only waits on the first ring's half of the load
                psb = [0, SSPLIT - 1, NSL] if len(load_plan[ch]) > 1 \
                    else [0, NSL]
                for k in range(len(psb) - 1):
                    s0, s1 = psb[k], psb[k + 1]
                    nc.vector.tensor_tensor(
                        out=P[:, s0:s1], in0=Xt[:, s0:s1],
                        in1=Xt[:, s0 + 1:s1 + 1], op=MIN)
                Q = pool.tile([NP, K, lw], f16, tag=f"Q{ch}")
                nc.vector.tensor_tensor(out=Q, in0=P[:, 0:K],
                                        in1=Xt[:, 4:SLOTS], op=MIN)
                V = pool.tile([NP, K, lw], f16, tag=f"V{ch}")
                nc.vector.tensor_tensor(out=V, in0=Q,
                                        in1=P[:, 2:K + 2], op=MIN)

                # horizontal 5-tap min, even shifts only (keeps 2x mode):
                # A = min(V, V<<2); B = min(A, A<<2) covers {0,2,4}.
                A = pool.tile([NP, K, lw - 2], f16, tag=f"A{ch}")
                nc.vector.tensor_tensor(out=A, in0=V[:, :, 0:lw - 2],
                                        in1=V[:, :, 2:lw], op=MIN)
                Bt = pool.tile([NP, K, lw - 4], f16, tag=f"B{ch}")
                nc.vector.tensor_tensor(out=Bt, in0=A[:, :, 0:lw - 4],
                                        in1=A[:, :, 2:lw - 2], op=MIN)

                # odd-parity part {1,3} = A<<1, materialized 4B-aligned by
                # a byte-addressed DMA copy so the final op runs at 2x.
                # The copy is done flat (one contiguous run per partition,
                # junk in the last column of each row, which is never read).
                A1r = None
                if a1_q1[ch]:
                    aw = lw - 2
                    A1 = pool.tile([NP, K * aw], f16, tag=f"A1{ch}")
                    L = K * aw - 1
                    Af = A.rearrange("p k w -> p (k w)")
                    nc.gpsimd.dma_start(out=A1[:, 0:L], in_=Af[:, 1:1 + L])
                    A1r = A1.rearrange("p (k w) -> p k w", k=K, w=aw)

                if pend is not None:
                    emit_final(pend)
                pend = (ch, A, Bt, A1r)
            emit_final(pend)
    nc.compile()
    return nc


def _get_program():
    if "nc" not in _cached:
        _cached["nc"] = _build_program()
    return _cached["nc"]


# stripe gather index: [PPI, SLOTS] padded-row index per (j, s)
_ROW_IDX = (K * np.arange(PPI)[:, None] + np.arange(SLOTS)[None, :])


def _stripe_core_input(x3: np.ndarray) -> np.ndarray:
    """[3,512,512] f16 -> host-striped flat input [IN_ELEMS] f16."""
    xp = np.zeros((IMGS, PAD_H, PAD_W), np.float16)
    xp[:, 2:2 + H, 2:2 + W] = x3
    stripes = np.zeros((NP, SLOTS, PAD_W), np.float16)
    stripes[:NP_DATA] = xp[:, _ROW_IDX, :].reshape(NP_DATA, SLOTS, PAD_W)
    parts = [
        stripes[:, :, c0:c0 + lw].reshape(-1)
        for c0, lw in zip(CHUNK_C0, LWS)
    ]
    return np.concatenate(parts)


def _out_pieces():
    pieces = []
    for ch in range(NCH):
        cw = CWS[ch]
        nsp = H_SPLITS[ch]
        bounds = [cw * t // nsp for t in range(nsp + 1)]
        for t in range(nsp):
            pieces.append((CHUNK_C0[ch] + bounds[t], bounds[t + 1] - bounds[t]))
    return pieces


_PIECES = None


def _unstripe_core_output(flat: np.ndarray) -> np.ndarray:
    """piece-blocked f16 output -> [3,512,512] f16."""
    global _PIECES
    if _PIECES is None:
        _PIECES = _out_pieces()
    stripes = np.empty((NP_DATA, K, W), np.float16)
    off = 0
    for col0, pw in _PIECES:
        blk = flat[off:off + NP * K * pw].reshape(NP, K, pw)
        stripes[:, :, col0:col0 + pw] = blk[:NP_DATA]
        off += NP * K * pw
    ys = stripes.reshape(IMGS, PPI, K, W)
    out = np.empty((IMGS, H, W), np.float16)
    full = (PPI - 1) * K  # 507 rows from full partitions
    out[:, :full] = ys[:, :PPI - 1].reshape(IMGS, full, W)
    out[:, full:] = ys[:, PPI - 1, :H - full]
    return out


def _run_on_hw(x24: np.ndarray, trace: bool = False):
    from concourse.bass_utils import run_bass_kernel_spmd
    nc = _get_program()
    x16 = x24.astype(np.float16)
    in_maps = [
        {"xs": _stripe_core_input(x16[IMGS * k:IMGS * (k + 1)])}
        for k in range(N_CORES)
    ]
    try:
        res = run_bass_kernel_spmd(nc, in_maps, list(range(N_CORES)),
                                   trace=trace)
    except Exception:
        import time
        time.sleep(5)
        res = run_bass_kernel_spmd(nc, in_maps, list(range(N_CORES)),
                                   trace=trace)
    out = np.stack([
        _unstripe_core_output(res.results[k]["ys"]) for k in range(N_CORES)
    ])
    return out.reshape(B, C, H, W).astype(np.float32), res


def _erode_reference_np(x: np.ndarray, se: np.ndarray) -> np.ndarray:
    """Generic fallback faithful to the kornia-style formula (numpy)."""
    kh, kw = se.shape
    ph, pw = kh // 2, kw // 2
    xpad = np.pad(x, ((0, 0), (0, 0), (ph, ph), (pw, pw)))
    out = None
    for r in range(kh):
        for c in range(kw):
            shifted = xpad[:, :, r:r + x.shape[2], c:c + x.shape[3]]
            bias = se[r, c] - 1.0
            val = shifted - bias if bias >= 0.0 else np.full_like(shifted, -bias)
            out = val if out is None else np.minimum(out, val)
    return out.astype(x.dtype)


def kernel(x, se):
    x = np.asarray(x, dtype=np.float32)
    se = np.asarray(se, dtype=np.float32)
    if se.shape != (5, 5) or not np.all(se == 1.0) or x.shape != (B, C, H, W):
        return _erode_reference_np(x, se)
    x24 = np.ascontiguousarray(x.reshape(B * C, H, W))
    out, _ = _run_on_hw(x24, trace=False)
    return out
